# revision 5
# baseline (speedup 1.0000x reference)
"""Differential cross-attention Trainium2 kernel (8 NeuronCores).

Sharding: 8 cores = (batch b = c//2) x (query half = c%2); each core does
all 8 heads for its 512 queries x 1024 keys. All matmuls bf16 (fp32 PE
matmul is 4 cyc/row vs 1 for bf16).

Head-pair permuted projections: output dims reordered so pair block hp
holds [head hp dims (parts 0:64) | head hp+4 dims (parts 64:128)]. This
lets the two score matmuls of a pair run row-tiled concurrently
(tile_position (0,0)/(64,0)) and makes V pair slices contiguous lhsT.

PV uses V as the stationary operand so outputs land transposed [dv, q]
(no PE transposes): per (hp, m) two slots:
  slot1 (col-tiled concurrent): [v1(64c)@(0,0) rhs=u1 | ones@(0,64)
    rhs=u1 -> S1 | ones@(0,96) rhs=u2 -> S2]
  slot2: [v1|v2](128c) rhs=u2
Per-q combine scalars (1+a)/S1, a*lam/S2, 1/S2 broadcast to 128
partitions via K=1 ones matmuls; combine on DVE.
"""
import sys
sys.path.insert(0, "/opt/trn_rl_repo")
import numpy as np

DIM = 512
H = 8
HD = 64
NQC = 512
NKV = 1024
MAX_DIST = 128
LAMBDA_INIT = 0.8
N_CORES = 8
SCALE = HD ** -0.5

_COMPILED = {}


def _build(reps=1):
    import concourse.bacc as bacc
    import concourse.mybir as mybir
    from concourse.tile import TileContext

    f32 = mybir.dt.float32
    bf16 = mybir.dt.bfloat16
    AF = mybir.ActivationFunctionType
    OP = mybir.AluOpType
    nc = bacc.Bacc("TRN2", target_bir_lowering=False, debug=False,
                   num_devices=N_CORES)

    xq_T = nc.dram_tensor("xq_T", [DIM, NQC], bf16, kind="ExternalInput")
    xkv_T = nc.dram_tensor("xkv_T", [DIM, NKV], bf16, kind="ExternalInput")
    wq = nc.dram_tensor("wq", [DIM, DIM], bf16, kind="ExternalInput")
    wk = nc.dram_tensor("wk", [DIM, DIM], bf16, kind="ExternalInput")
    wv = nc.dram_tensor("wv", [DIM, DIM], bf16, kind="ExternalInput")
    wp = nc.dram_tensor("wp", [DIM, DIM], bf16, kind="ExternalInput")
    bq = nc.dram_tensor("bq", [128, 4], f32, kind="ExternalInput")
    bk = nc.dram_tensor("bk", [128, 4], f32, kind="ExternalInput")
    bv = nc.dram_tensor("bv", [128, DIM], f32, kind="ExternalInput")
    bp = nc.dram_tensor("bp", [128, 4], f32, kind="ExternalInput")
    al1 = nc.dram_tensor("al1", [1, NQC], f32, kind="ExternalInput")
    alam = nc.dram_tensor("alam", [1, 4, NQC], f32, kind="ExternalInput")
    biasE = nc.dram_tensor("biasE", [4, 8, 2, 128, NQC], bf16,
                           kind="ExternalInput")
    out_T = nc.dram_tensor("out_T", [DIM, NQC], f32, kind="ExternalOutput")

    with TileContext(nc) as tc:
      for _rep in range(reps):
        with (
            tc.tile_pool(name="const", bufs=1) as cpool,
            tc.tile_pool(name="work", bufs=1) as wpool,
            tc.tile_pool(name="stream", bufs=6) as spool,
            tc.tile_pool(name="biasp", bufs=4) as bpool,
            tc.tile_pool(name="upool", bufs=6) as upool,
            tc.tile_pool(name="psproj", bufs=2, space="PSUM") as ppool,
            tc.tile_pool(name="pssc", bufs=1, space="PSUM") as scpool,
            tc.tile_pool(name="pspv", bufs=2, space="PSUM") as pvpool,
        ):
            wq_t = cpool.tile([128, 4, DIM], bf16, tag="wq")
            wk_t = cpool.tile([128, 4, DIM], bf16, tag="wk")
            wv_t = cpool.tile([128, 4, DIM], bf16, tag="wv")
            wp_t = cpool.tile([128, 4, DIM], bf16, tag="wp")
            for w_t, w in ((wq_t, wq), (wk_t, wk), (wv_t, wv), (wp_t, wp)):
                for c in range(4):
                    nc.sync.dma_start(
                        out=w_t[:, c, :],
                        in_=w[:].rearrange("(c p) o -> p c o", p=128)[:, c, :])
            bq_t = cpool.tile([128, 4], f32, tag="bq")
            bk_t = cpool.tile([128, 4], f32, tag="bk")
            bv_t = cpool.tile([128, DIM], f32, tag="bv")
            bp_t = cpool.tile([128, 4], f32, tag="bp")
            al1_t = cpool.tile([1, NQC], f32, tag="al1")
            alam_t = cpool.tile([1, 4, NQC], f32, tag="alam")
            for t, src in ((bq_t, bq), (bk_t, bk), (bv_t, bv), (bp_t, bp),
                           (al1_t, al1), (alam_t, alam)):
                nc.sync.dma_start(out=t[:], in_=src[:])
            ones_col = cpool.tile([128, 1], bf16, tag="ones")
            nc.vector.memset(ones_col[:], 1.0)

            xq_t = wpool.tile([128, 4, NQC], bf16, tag="xq")
            xkv_t = wpool.tile([128, 4, NKV], bf16, tag="xkv")
            for c in range(4):
                nc.sync.dma_start(out=xq_t[:, c, :],
                                  in_=xq_T[:].rearrange("(c p) n -> p c n", p=128)[:, c, :])
                nc.sync.dma_start(out=xkv_t[:, c, :],
                                  in_=xkv_T[:].rearrange("(c p) n -> p c n", p=128)[:, c, :])

            qp = wpool.tile([128, 4, NQC], bf16, tag="qp")
            kp = wpool.tile([128, 4, NKV], bf16, tag="kp")
            vp = wpool.tile([128, 8, DIM], bf16, tag="vp")
            for t in range(4):
                ps = ppool.tile([128, NQC], f32, tag="proj")
                for c in range(4):
                    nc.tensor.matmul(ps[:], lhsT=wq_t[:, c, 128 * t:128 * (t + 1)],
                                     rhs=xq_t[:, c, :], start=(c == 0), stop=(c == 3))
                nc.vector.tensor_scalar(out=qp[:, t, :], in0=ps[:],
                                        scalar1=bq_t[:, t:t + 1], scalar2=None,
                                        op0=OP.add)
            for t in range(4):
                for kh in range(2):
                    ps = ppool.tile([128, NQC], f32, tag="proj")
                    for c in range(4):
                        nc.tensor.matmul(
                            ps[:], lhsT=wk_t[:, c, 128 * t:128 * (t + 1)],
                            rhs=xkv_t[:, c, 512 * kh:512 * (kh + 1)],
                            start=(c == 0), stop=(c == 3))
                    nc.vector.tensor_scalar(out=kp[:, t, 512 * kh:512 * (kh + 1)],
                                            in0=ps[:], scalar1=bk_t[:, t:t + 1],
                                            scalar2=None, op0=OP.add)
            for m in range(8):
                ps = ppool.tile([128, NQC], f32, tag="proj")
                for c in range(4):
                    nc.tensor.matmul(ps[:], lhsT=xkv_t[:, c, 128 * m:128 * (m + 1)],
                                     rhs=wv_t[:, c, :], start=(c == 0), stop=(c == 3))
                nc.vector.tensor_tensor(out=vp[:, m, :], in0=ps[:], in1=bv_t[:],
                                        op=OP.add)

            xcat = wpool.tile([128, 4, NQC], bf16, tag="xcat")
            for hp in range(4):
                pv1 = pvpool.tile([128, NQC], f32, tag="pv1")
                pv2 = pvpool.tile([128, NQC], f32, tag="pv2")
                for m in range(8):
                    btp = bpool.tile([128, 2, NQC], bf16, tag="btp")
                    nc.sync.dma_start(
                        out=btp[:],
                        in_=biasE[hp, m, :, :, :].rearrange("t p n -> p t n"))
                    s1 = scpool.tile([128, NQC], f32, tag="s1")
                    s2 = scpool.tile([128, NQC], f32, tag="s2")
                    nc.tensor.matmul(
                        s1[:], lhsT=kp[0:64, hp, 128 * m:128 * (m + 1)],
                        rhs=qp[0:64, hp, :], start=True, stop=True)
                    nc.tensor.matmul(
                        s2[:], lhsT=kp[64:128, hp, 128 * m:128 * (m + 1)],
                        rhs=qp[64:128, hp, :], start=True, stop=True)
                    u1 = upool.tile([128, NQC], bf16, tag="u1")
                    nc.scalar.activation(u1[:], s1[:], AF.Exp)
                    u1b = upool.tile([128, NQC], bf16, tag="u1b")
                    nc.vector.tensor_tensor(out=u1b[:], in0=u1[:],
                                            in1=btp[:, 0, :], op=OP.mult)
                    u2 = upool.tile([128, NQC], bf16, tag="u2")
                    nc.scalar.activation(u2[:], s2[:], AF.Exp)
                    u2b = upool.tile([128, NQC], bf16, tag="u2b")
                    nc.vector.tensor_tensor(out=u2b[:], in0=u2[:],
                                            in1=btp[:, 1, :], op=OP.mult)
                    st, sp = (m == 0), (m == 7)
                    nc.tensor.matmul(pv1[0:64, :],
                                     lhsT=vp[:, m, 128 * hp:128 * hp + 64],
                                     rhs=u1b[:], start=st, stop=sp)
                    nc.tensor.matmul(pv1[64:65, :], lhsT=ones_col[:],
                                     rhs=u1b[:], start=st, stop=sp)
                    nc.tensor.matmul(pv1[96:97, :], lhsT=ones_col[:],
                                     rhs=u2b[:], start=st, stop=sp,
                                     tile_position=(0, 96))
                    nc.tensor.matmul(pv2[:, :],
                                     lhsT=vp[:, m, 128 * hp:128 * (hp + 1)],
                                     rhs=u2b[:], start=st, stop=sp)
                rec1 = spool.tile([1, NQC], f32, tag="rec1")
                nc.vector.reciprocal(rec1[:], pv1[64:65, :])
                rec2 = spool.tile([1, NQC], f32, tag="rec2")
                nc.vector.reciprocal(rec2[:], pv1[96:97, :])
                c1 = spool.tile([1, NQC], bf16, tag="c1")
                nc.vector.tensor_tensor(out=c1[:], in0=rec1[:], in1=al1_t[:],
                                        op=OP.mult)
                c2 = spool.tile([1, NQC], bf16, tag="c2")
                nc.vector.tensor_tensor(out=c2[:], in0=rec2[:],
                                        in1=alam_t[:, hp, :], op=OP.mult)
                c3 = spool.tile([1, NQC], bf16, tag="c3")
                nc.vector.tensor_copy(out=c3[:], in_=rec2[:])
                B1 = spool.tile([128, NQC], bf16, tag="B1")
                nc.gpsimd.partition_broadcast(B1[:], c1[:])
                B2 = spool.tile([128, NQC], bf16, tag="B2")
                nc.gpsimd.partition_broadcast(B2[:], c2[:])
                B3 = spool.tile([128, NQC], bf16, tag="B3")
                nc.gpsimd.partition_broadcast(B3[:], c3[:])
                t1 = spool.tile([64, NQC], f32, tag="t1")
                nc.vector.tensor_tensor(out=t1[:], in0=pv1[0:64, :],
                                        in1=B1[0:64, :], op=OP.mult)
                t2 = spool.tile([64, NQC], f32, tag="t2")
                nc.vector.tensor_tensor(out=t2[:], in0=pv2[0:64, :],
                                        in1=B2[0:64, :], op=OP.mult)
                nc.vector.tensor_tensor(out=xcat[0:64, hp, :], in0=t1[:],
                                        in1=t2[:], op=OP.subtract)
                nc.vector.tensor_tensor(out=xcat[64:128, hp, :],
                                        in0=pv2[64:128, :], in1=B3[64:128, :],
                                        op=OP.mult)

            for t in range(4):
                ps = ppool.tile([128, NQC], f32, tag="proj")
                for c in range(4):
                    nc.tensor.matmul(ps[:], lhsT=wp_t[:, c, 128 * t:128 * (t + 1)],
                                     rhs=xcat[:, c, :], start=(c == 0), stop=(c == 3))
                ot = wpool.tile([128, NQC], f32, tag="otile")
                nc.vector.tensor_scalar(out=ot[:], in0=ps[:],
                                        scalar1=bp_t[:, t:t + 1], scalar2=None,
                                        op0=OP.add)
                nc.sync.dma_start(
                    out=out_T[:].rearrange("(c p) n -> p c n", p=128)[:, t, :],
                    in_=ot[:])
    nc.compile()
    return nc


def _get_kernel(reps=1):
    key = f"k{reps}"
    if key not in _COMPILED:
        _COMPILED[key] = _build(reps)
    return _COMPILED[key]


def _bf16(a):
    import ml_dtypes
    return np.ascontiguousarray(np.asarray(a, dtype=np.float32).astype(ml_dtypes.bfloat16))


_PERM = np.concatenate(
    [np.r_[64 * hp:64 * (hp + 1), 64 * (hp + 4):64 * (hp + 5)] for hp in range(4)])


def _prep_inputs(x_q, x_kv, coords_q, coords_k, alpha_map,
                 Wq, bq, Wk, bk, Wv, bv,
                 lambda_q1, lambda_k1, lambda_q2, lambda_k2,
                 rpe_table, Wp, bp):
    x_q = np.asarray(x_q, dtype=np.float32)
    x_kv = np.asarray(x_kv, dtype=np.float32)
    coords_q = np.asarray(coords_q)
    coords_k = np.asarray(coords_k)
    alpha_map = np.asarray(alpha_map, dtype=np.float32)
    rpe = np.asarray(rpe_table, dtype=np.float32)

    lam1 = np.exp(np.sum(np.asarray(lambda_q1) * np.asarray(lambda_k1), axis=-1))
    lam2 = np.exp(np.sum(np.asarray(lambda_q2) * np.asarray(lambda_k2), axis=-1))
    lam = (lam1 - lam2 + LAMBDA_INIT).astype(np.float32)  # [4]

    P = _PERM
    wq_l = _bf16((np.asarray(Wq, np.float32).T * SCALE)[:, P])
    wk_l = _bf16(np.asarray(Wk, np.float32).T[:, P])
    wv_l = _bf16(np.asarray(Wv, np.float32).T[:, P])
    wp_l = _bf16(np.asarray(Wp, np.float32).T[P, :])
    bq_l = np.ascontiguousarray(
        (np.asarray(bq, np.float32) * SCALE)[P].reshape(4, 128).T)
    bk_l = np.ascontiguousarray(np.asarray(bk, np.float32)[P].reshape(4, 128).T)
    bv_l = np.ascontiguousarray(
        np.tile(np.asarray(bv, np.float32)[P][None, :], (128, 1)))
    bp_l = np.ascontiguousarray(np.asarray(bp, np.float32).reshape(4, 128).T)

    in_maps = []
    for c in range(N_CORES):
        b, qh = divmod(c, 2)
        qsl = slice(qh * NQC, (qh + 1) * NQC)
        cq = coords_q[b, qsl]
        ck = coords_k[b]
        rel = cq[:, None, :] - ck[None, :, :] + MAX_DIST
        rel = np.clip(rel, 0, 2 * MAX_DIST)
        idx = rel[..., 0] * (2 * MAX_DIST + 1) + rel[..., 1]
        bias = np.exp(rpe[idx])                                # [512q, 1024k, 8]
        biasT = bias.transpose(2, 1, 0)                        # [8, 1024k, 512q]
        biasT = np.ascontiguousarray(
            biasT.reshape(2, 4, 8, 128, NQC).transpose(1, 2, 0, 3, 4))
        alpha = alpha_map[b, qsl, 0]                           # [512]
        in_maps.append({
            "xq_T": _bf16(x_q[b, qsl].T),
            "xkv_T": _bf16(x_kv[b].T),
            "wq": wq_l, "wk": wk_l, "wv": wv_l, "wp": wp_l,
            "bq": bq_l, "bk": bk_l, "bv": bv_l, "bp": bp_l,
            "al1": np.ascontiguousarray((1.0 + alpha).reshape(1, NQC)),
            "alam": np.ascontiguousarray(
                (alpha[None, :] * lam[:, None]).reshape(1, 4, NQC)),
            "biasE": _bf16(biasT),
        })
    return in_maps


def kernel(x_q, x_kv, coords_q, coords_k, alpha_map,
           Wq, bq, Wk, bk, Wv, bv,
           lambda_q1, lambda_k1, lambda_q2, lambda_k2,
           rpe_table, Wp, bp):
    from concourse.bass_utils import run_bass_kernel_spmd

    nc = _get_kernel()
    in_maps = _prep_inputs(x_q, x_kv, coords_q, coords_k, alpha_map,
                           Wq, bq, Wk, bk, Wv, bv,
                           lambda_q1, lambda_k1, lambda_q2, lambda_k2,
                           rpe_table, Wp, bp)
    res = run_bass_kernel_spmd(nc, in_maps, list(range(N_CORES)))
    B = np.asarray(x_q).shape[0]
    out = np.zeros((B, 2 * NQC, DIM), dtype=np.float32)
    for c in range(N_CORES):
        b, qh = divmod(c, 2)
        out[b, qh * NQC:(qh + 1) * NQC] = res.results[c]["out_T"].T
    return out


# revision 9
# speedup vs baseline: 1.3264x; 1.3264x over previous
"""Differential cross-attention Trainium2 kernel (8 NeuronCores).

Sharding: 8 cores = (batch b = c//2) x (query half = c%2); each core does
all 8 heads for its 512 queries x 1024 keys. All matmuls bf16 (fp32 PE
matmul is 4 cyc/row vs 1 for bf16).

Head-pair permuted projections: output dims reordered so pair block hp
holds [head hp dims (parts 0:64) | head hp+4 dims (parts 64:128)]. This
lets the two score matmuls of a pair run row-tiled concurrently
(tile_position (0,0)/(64,0)) and makes V pair slices contiguous lhsT.

PV uses V as the stationary operand so outputs land transposed [dv, q]
(no PE transposes): per (hp, m) two slots:
  slot1 (col-tiled concurrent): [v1(64c)@(0,0) rhs=u1 | ones@(0,64)
    rhs=u1 -> S1 | ones@(0,96) rhs=u2 -> S2]
  slot2: [v1|v2](128c) rhs=u2
Per-q combine scalars (1+a)/S1, a*lam/S2, 1/S2 broadcast to 128
partitions via K=1 ones matmuls; combine on DVE.
"""
import sys
sys.path.insert(0, "/opt/trn_rl_repo")
import numpy as np

DIM = 512
H = 8
HD = 64
NQC = 512
NKV = 1024
MAX_DIST = 128
LAMBDA_INIT = 0.8
N_CORES = 8
SCALE = HD ** -0.5

_COMPILED = {}


def _build(reps=1):
    import concourse.bacc as bacc
    import concourse.mybir as mybir
    from concourse.tile import TileContext

    f32 = mybir.dt.float32
    bf16 = mybir.dt.bfloat16
    AF = mybir.ActivationFunctionType
    OP = mybir.AluOpType
    nc = bacc.Bacc("TRN2", target_bir_lowering=False, debug=False,
                   num_devices=N_CORES)

    xq_T = nc.dram_tensor("xq_T", [DIM, NQC], bf16, kind="ExternalInput")
    xkv_T = nc.dram_tensor("xkv_T", [DIM, NKV], bf16, kind="ExternalInput")
    wq = nc.dram_tensor("wq", [DIM, DIM], bf16, kind="ExternalInput")
    wk = nc.dram_tensor("wk", [DIM, DIM], bf16, kind="ExternalInput")
    wv = nc.dram_tensor("wv", [DIM, DIM], bf16, kind="ExternalInput")
    wp = nc.dram_tensor("wp", [DIM, DIM], bf16, kind="ExternalInput")
    bq = nc.dram_tensor("bq", [128, 4], f32, kind="ExternalInput")
    bk = nc.dram_tensor("bk", [128, 4], f32, kind="ExternalInput")
    bv = nc.dram_tensor("bv", [128, DIM], f32, kind="ExternalInput")
    bp = nc.dram_tensor("bp", [128, 4], f32, kind="ExternalInput")
    al1 = nc.dram_tensor("al1", [1, NQC], f32, kind="ExternalInput")
    alam = nc.dram_tensor("alam", [1, 4, NQC], f32, kind="ExternalInput")
    biasE = nc.dram_tensor("biasE", [4, 8, 128, 2, NQC], bf16,
                           kind="ExternalInput")
    out_T = nc.dram_tensor("out_T", [DIM, NQC], f32, kind="ExternalOutput")

    with TileContext(nc) as tc:
      for _rep in range(reps):
        with (
            tc.tile_pool(name="const", bufs=1) as cpool,
            tc.tile_pool(name="work", bufs=1) as wpool,
            tc.tile_pool(name="stream", bufs=6) as spool,
            tc.tile_pool(name="biasp", bufs=4) as bpool,
            tc.tile_pool(name="upool", bufs=6) as upool,
            tc.tile_pool(name="psproj", bufs=2, space="PSUM") as ppool,
            tc.tile_pool(name="pssc", bufs=2, space="PSUM") as scpool,
            tc.tile_pool(name="pspv", bufs=1, space="PSUM") as pvpool,
        ):
            wq_t = cpool.tile([128, 4, DIM], bf16, tag="wq")
            wk_t = cpool.tile([128, 4, DIM], bf16, tag="wk")
            wv_t = cpool.tile([128, 4, DIM], bf16, tag="wv")
            wp_t = cpool.tile([128, 4, DIM], bf16, tag="wp")
            def _load_w(w_t, w, eng):
                eng.dma_start(out=w_t[:],
                              in_=w[:].rearrange("(c p) o -> p c o", p=128))
            _load_w(wq_t, wq, nc.sync)
            bq_t = cpool.tile([128, 4], f32, tag="bq")
            bk_t = cpool.tile([128, 4], f32, tag="bk")
            bv_t = cpool.tile([128, DIM], f32, tag="bv")
            bp_t = cpool.tile([128, 4], f32, tag="bp")
            al1_t = cpool.tile([1, NQC], f32, tag="al1")
            alam_t = cpool.tile([1, 4, NQC], f32, tag="alam")
            for t, src in ((bq_t, bq), (bk_t, bk), (bv_t, bv), (bp_t, bp),
                           (al1_t, al1), (alam_t, alam)):
                nc.gpsimd.dma_start(out=t[:], in_=src[:])
            ones_col = cpool.tile([128, 1], bf16, tag="ones")
            nc.vector.memset(ones_col[:], 1.0)

            xq_t = wpool.tile([128, 4, NQC], bf16, tag="xq")
            xkv_t = wpool.tile([128, 4, NKV], bf16, tag="xkv")
            nc.sync.dma_start(out=xq_t[:],
                              in_=xq_T[:].rearrange("(c p) n -> p c n", p=128))
            nc.gpsimd.dma_start(out=xkv_t[:],
                                in_=xkv_T[:].rearrange("(c p) n -> p c n", p=128))
            _load_w(wk_t, wk, nc.gpsimd)
            _load_w(wv_t, wv, nc.gpsimd)
            _load_w(wp_t, wp, nc.gpsimd)

            qp = wpool.tile([128, 4, NQC], bf16, tag="qp")
            kp = wpool.tile([128, 4, NKV], bf16, tag="kp")
            vp = wpool.tile([128, 8, DIM], bf16, tag="vp")

            def qproj(t):
                ps = ppool.tile([128, NQC], f32, tag="proj")
                for c in range(4):
                    nc.tensor.matmul(ps[:], lhsT=wq_t[:, c, 128 * t:128 * (t + 1)],
                                     rhs=xq_t[:, c, :], start=(c == 0), stop=(c == 3))
                nc.scalar.activation(qp[:, t, :], ps[:], AF.Identity,
                                     bias=bq_t[:, t:t + 1])

            def kproj(t):
                for kh in range(2):
                    ps = ppool.tile([128, NQC], f32, tag="proj")
                    for c in range(4):
                        nc.tensor.matmul(
                            ps[:], lhsT=wk_t[:, c, 128 * t:128 * (t + 1)],
                            rhs=xkv_t[:, c, 512 * kh:512 * (kh + 1)],
                            start=(c == 0), stop=(c == 3))
                    nc.vector.tensor_scalar(out=kp[:, t, 512 * kh:512 * (kh + 1)],
                                            in0=ps[:], scalar1=bk_t[:, t:t + 1],
                                            scalar2=None, op0=OP.add)

            def vproj(m):
                ps = ppool.tile([128, NQC], f32, tag="proj")
                for c in range(4):
                    nc.tensor.matmul(ps[:], lhsT=xkv_t[:, c, 128 * m:128 * (m + 1)],
                                     rhs=wv_t[:, c, :], start=(c == 0), stop=(c == 3))
                nc.vector.tensor_tensor(out=vp[:, m, :], in0=ps[:], in1=bv_t[:],
                                        op=OP.add)

            qproj(0)
            kproj(0)

            xcat = wpool.tile([128, 4, NQC], bf16, tag="xcat")
            pvq = []   # deferred PV matmul emitters (software pipeline, lag 1)

            def emit_pv(fn):
                pvq.append(fn)

            def drain_pv():
                while pvq:
                    pvq.pop(0)()

            for hp in range(4):
                pv1 = pvpool.tile([128, NQC], f32, tag="pv1")
                pv2 = pvpool.tile([128, NQC], f32, tag="pv2")
                for m in range(8):
                    btp = bpool.tile([128, 2, NQC], bf16, tag="btp")
                    nc.sync.dma_start(out=btp[:], in_=biasE[hp, m])
                    if hp == 0:
                        vproj(m)
                    if hp < 3 and m == 4:
                        kproj(hp + 1)
                    if hp < 3 and m == 6:
                        qproj(hp + 1)
                    s12 = scpool.tile([128, 2, NQC], f32, tag="s12")
                    nc.tensor.matmul(
                        s12[:, 0, :], lhsT=kp[0:64, hp, 128 * m:128 * (m + 1)],
                        rhs=qp[0:64, hp, :], start=True, stop=True)
                    nc.tensor.matmul(
                        s12[:, 1, :], lhsT=kp[64:128, hp, 128 * m:128 * (m + 1)],
                        rhs=qp[64:128, hp, :], start=True, stop=True)
                    u12 = upool.tile([128, 2, NQC], bf16, tag="u12")
                    nc.scalar.activation(u12[:], s12[:], AF.Exp)
                    u12b = upool.tile([128, 2, NQC], bf16, tag="u12b")
                    meng = nc.gpsimd if (m % 4 == 3) else nc.vector
                    meng.tensor_tensor(out=u12b[:], in0=u12[:],
                                       in1=btp[:], op=OP.mult)
                    st, sp = (m == 0), (m == 7)

                    def pv_mms(pv1=pv1, pv2=pv2, u12b=u12b, m=m, hp=hp,
                               st=st, sp=sp):
                        nc.tensor.matmul(pv1[0:64, :],
                                         lhsT=vp[:, m, 128 * hp:128 * hp + 64],
                                         rhs=u12b[:, 0, :], start=st, stop=sp)
                        nc.tensor.matmul(pv1[64:65, :], lhsT=ones_col[:],
                                         rhs=u12b[:, 0, :], start=st, stop=sp)
                        nc.tensor.matmul(pv1[96:97, :], lhsT=ones_col[:],
                                         rhs=u12b[:, 1, :], start=st, stop=sp,
                                         tile_position=(0, 96))
                        nc.tensor.matmul(pv2[:, :],
                                         lhsT=vp[:, m, 128 * hp:128 * (hp + 1)],
                                         rhs=u12b[:, 1, :], start=st, stop=sp)
                    emit_pv(pv_mms)
                    if not (hp == 0 and m == 0):
                        drain_pv()  # keeps exactly one PV set pending
                drain_pv()
                rec1 = spool.tile([1, NQC], f32, tag="rec1")
                nc.vector.reciprocal(rec1[:], pv1[64:65, :])
                rec2 = spool.tile([1, NQC], f32, tag="rec2")
                nc.vector.reciprocal(rec2[:], pv1[96:97, :])
                c1 = spool.tile([1, NQC], bf16, tag="c1")
                nc.vector.tensor_tensor(out=c1[:], in0=rec1[:], in1=al1_t[:],
                                        op=OP.mult)
                c2 = spool.tile([1, NQC], bf16, tag="c2")
                nc.vector.tensor_tensor(out=c2[:], in0=rec2[:],
                                        in1=alam_t[:, hp, :], op=OP.mult)
                c3 = spool.tile([1, NQC], bf16, tag="c3")
                nc.vector.tensor_copy(out=c3[:], in_=rec2[:])
                B1 = spool.tile([128, NQC], bf16, tag="B1")
                nc.gpsimd.partition_broadcast(B1[:], c1[:])
                B2 = spool.tile([128, NQC], bf16, tag="B2")
                nc.gpsimd.partition_broadcast(B2[:], c2[:])
                B3 = spool.tile([128, NQC], bf16, tag="B3")
                nc.gpsimd.partition_broadcast(B3[:], c3[:])
                t1 = spool.tile([64, NQC], f32, tag="t1")
                nc.vector.tensor_tensor(out=t1[:], in0=pv1[0:64, :],
                                        in1=B1[0:64, :], op=OP.mult)
                t2 = spool.tile([64, NQC], f32, tag="t2")
                nc.vector.tensor_tensor(out=t2[:], in0=pv2[0:64, :],
                                        in1=B2[0:64, :], op=OP.mult)
                nc.vector.tensor_tensor(out=xcat[0:64, hp, :], in0=t1[:],
                                        in1=t2[:], op=OP.subtract)
                nc.vector.tensor_tensor(out=xcat[64:128, hp, :],
                                        in0=pv2[64:128, :], in1=B3[64:128, :],
                                        op=OP.mult)

            for t in range(4):
                ps = ppool.tile([128, NQC], f32, tag="proj")
                for c in range(4):
                    nc.tensor.matmul(ps[:], lhsT=wp_t[:, c, 128 * t:128 * (t + 1)],
                                     rhs=xcat[:, c, :], start=(c == 0), stop=(c == 3))
                ot = wpool.tile([128, NQC], f32, tag="otile")
                nc.scalar.activation(ot[:], ps[:], AF.Identity,
                                     bias=bp_t[:, t:t + 1])
                oeng = nc.scalar if (t % 2 == 0) else nc.sync
                oeng.dma_start(
                    out=out_T[:].rearrange("(c p) n -> p c n", p=128)[:, t, :],
                    in_=ot[:])
    nc.compile()
    return nc


def _get_kernel(reps=1):
    key = f"k{reps}"
    if key not in _COMPILED:
        _COMPILED[key] = _build(reps)
    return _COMPILED[key]


def _bf16(a):
    import ml_dtypes
    return np.ascontiguousarray(np.asarray(a, dtype=np.float32).astype(ml_dtypes.bfloat16))


_PERM = np.concatenate(
    [np.r_[64 * hp:64 * (hp + 1), 64 * (hp + 4):64 * (hp + 5)] for hp in range(4)])


def _prep_inputs(x_q, x_kv, coords_q, coords_k, alpha_map,
                 Wq, bq, Wk, bk, Wv, bv,
                 lambda_q1, lambda_k1, lambda_q2, lambda_k2,
                 rpe_table, Wp, bp):
    x_q = np.asarray(x_q, dtype=np.float32)
    x_kv = np.asarray(x_kv, dtype=np.float32)
    coords_q = np.asarray(coords_q)
    coords_k = np.asarray(coords_k)
    alpha_map = np.asarray(alpha_map, dtype=np.float32)
    rpe = np.asarray(rpe_table, dtype=np.float32)

    lam1 = np.exp(np.sum(np.asarray(lambda_q1) * np.asarray(lambda_k1), axis=-1))
    lam2 = np.exp(np.sum(np.asarray(lambda_q2) * np.asarray(lambda_k2), axis=-1))
    lam = (lam1 - lam2 + LAMBDA_INIT).astype(np.float32)  # [4]

    P = _PERM
    wq_l = _bf16((np.asarray(Wq, np.float32).T * SCALE)[:, P])
    wk_l = _bf16(np.asarray(Wk, np.float32).T[:, P])
    wv_l = _bf16(np.asarray(Wv, np.float32).T[:, P])
    wp_l = _bf16(np.asarray(Wp, np.float32).T[P, :])
    bq_l = np.ascontiguousarray(
        (np.asarray(bq, np.float32) * SCALE)[P].reshape(4, 128).T)
    bk_l = np.ascontiguousarray(np.asarray(bk, np.float32)[P].reshape(4, 128).T)
    bv_l = np.ascontiguousarray(
        np.tile(np.asarray(bv, np.float32)[P][None, :], (128, 1)))
    bp_l = np.ascontiguousarray(np.asarray(bp, np.float32).reshape(4, 128).T)

    in_maps = []
    for c in range(N_CORES):
        b, qh = divmod(c, 2)
        qsl = slice(qh * NQC, (qh + 1) * NQC)
        cq = coords_q[b, qsl]
        ck = coords_k[b]
        rel = cq[:, None, :] - ck[None, :, :] + MAX_DIST
        rel = np.clip(rel, 0, 2 * MAX_DIST)
        idx = rel[..., 0] * (2 * MAX_DIST + 1) + rel[..., 1]
        bias = np.exp(rpe[idx])                                # [512q, 1024k, 8]
        biasT = bias.transpose(2, 1, 0)                        # [8, 1024k, 512q]
        # -> [hp, m, k%128, half, q] so the per-(hp,m) DMA is layout-direct
        biasT = np.ascontiguousarray(
            biasT.reshape(2, 4, 8, 128, NQC).transpose(1, 2, 3, 0, 4))
        alpha = alpha_map[b, qsl, 0]                           # [512]
        in_maps.append({
            "xq_T": _bf16(x_q[b, qsl].T),
            "xkv_T": _bf16(x_kv[b].T),
            "wq": wq_l, "wk": wk_l, "wv": wv_l, "wp": wp_l,
            "bq": bq_l, "bk": bk_l, "bv": bv_l, "bp": bp_l,
            "al1": np.ascontiguousarray((1.0 + alpha).reshape(1, NQC)),
            "alam": np.ascontiguousarray(
                (alpha[None, :] * lam[:, None]).reshape(1, 4, NQC)),
            "biasE": _bf16(biasT),
        })
    return in_maps


def kernel(x_q, x_kv, coords_q, coords_k, alpha_map,
           Wq, bq, Wk, bk, Wv, bv,
           lambda_q1, lambda_k1, lambda_q2, lambda_k2,
           rpe_table, Wp, bp):
    from concourse.bass_utils import run_bass_kernel_spmd

    nc = _get_kernel()
    in_maps = _prep_inputs(x_q, x_kv, coords_q, coords_k, alpha_map,
                           Wq, bq, Wk, bk, Wv, bv,
                           lambda_q1, lambda_k1, lambda_q2, lambda_k2,
                           rpe_table, Wp, bp)
    res = run_bass_kernel_spmd(nc, in_maps, list(range(N_CORES)))
    B = np.asarray(x_q).shape[0]
    out = np.zeros((B, 2 * NQC, DIM), dtype=np.float32)
    for c in range(N_CORES):
        b, qh = divmod(c, 2)
        out[b, qh * NQC:(qh + 1) * NQC] = res.results[c]["out_T"].T
    return out
